# revision 5
# baseline (speedup 1.0000x reference)
"""MoE routing kernel for Trainium2, expert-parallel over 8 NeuronCores.

Model (B,S,D,H,E,K = 4,2048,1024,4096,8,2):
    routing = softmax(x @ Wr + br); top-2 renormalized -> dense gate [N,E]
    out = sum_e gate[:,e] * (relu(x @ W1[e] + b1[e]) @ W2[e] + b2[e])

Sharding: expert-parallel — core e owns expert e (W1[e], b1[e], W2[e]).
Every core computes the full router (fp32, so top-2 selection matches the
fp32 reference bit-for-bit up to near-ties) and its own expert's gated
partial output over all tokens; the host sums the 8 partials and adds the
gate-weighted b2 term (sum_e gate[:,e] * b2[e] == gate @ b2).

On-device pipeline per 512-token group, all matmuls on TensorE:
  router : logits[tok,8]  = x_f32 @ Wr (fp32 matmul, N=8)  + top-2 gate math
  MM1    : hT[m][h128,tok512] = relu(sum_k W1[k,m].T @ xT[k] + b1)  (bf16)
  MM2    : y[tok128,d512]     = sum_m hT[m][:,t].T @ W2[m]          (bf16)
  scale  : out = y * gate[:,e] (per-partition scalar on ScalarE), DMA out

FFN inputs are cast to bf16 (weights on host, x on chip); accumulation is
fp32 in PSUM. Router stays fp32 end-to-end.
"""

import sys

for _p in ("/opt/trn_rl_repo",):
    if _p not in sys.path:
        sys.path.insert(0, _p)

from contextlib import ExitStack

import ml_dtypes
import numpy as np

import concourse.bass as bass
import concourse.mybir as mybir
import concourse.tile as tile
from concourse.bass_utils import run_bass_kernel_spmd

BF16 = ml_dtypes.bfloat16
F32 = mybir.dt.float32
BF = mybir.dt.bfloat16
AF = mybir.ActivationFunctionType
ALU = mybir.AluOpType
AX = mybir.AxisListType

B, S, D, H, E = 4, 2048, 1024, 4096, 8
NTOK = B * S          # 8192 tokens
GT = 512              # tokens per group
NG = NTOK // GT       # 16 groups
KC = D // 128         # 8 contraction chunks for MM1 / router
MC = H // 128         # 32 H chunks
NSUB = GT // 128      # 4 token subtiles per group
DHALF = D // 512      # 2 output halves for MM2 (PSUM bank = 512 fp32)

_CACHE: dict = {}


def _build_nc() -> bass.Bass:
    nc = bass.Bass()
    xf = nc.declare_dram_parameter("xf", [KC, 128, NTOK], F32, isOutput=False)
    w1 = nc.declare_dram_parameter("w1", [128, MC, KC * 128], BF, isOutput=False)
    w2 = nc.declare_dram_parameter("w2", [128, MC * D], BF, isOutput=False)
    wr = nc.declare_dram_parameter("wr", [128, KC * E], F32, isOutput=False)
    brt = nc.declare_dram_parameter("brt", [128, E], F32, isOutput=False)
    b1 = nc.declare_dram_parameter("b1", [128, MC], F32, isOutput=False)
    gsel = nc.declare_dram_parameter("gsel", [128, E], F32, isOutput=False)
    outy = nc.declare_dram_parameter("outy", [NTOK, D], F32, isOutput=True)
    gateo = nc.declare_dram_parameter("gateo", [NTOK, E], F32, isOutput=True)

    with ExitStack() as ctx:
        tc = ctx.enter_context(tile.TileContext(nc))
        cpool = ctx.enter_context(tc.tile_pool(name="const", bufs=1))
        w2_sb = cpool.tile([128, MC * D], BF, name="w2sb")
        for m in range(MC):
            nc.sync.dma_start(w2_sb[:, m * D : (m + 1) * D], w2[:, m * D : (m + 1) * D])
        wr_sb = cpool.tile([128, KC * E], F32, name="wrsb")
        nc.sync.dma_start(wr_sb[:], wr[:])
        br_sb = cpool.tile([128, E], F32, name="brsb")
        nc.sync.dma_start(br_sb[:], brt[:])
        b1_sb = cpool.tile([128, MC], F32, name="b1sb")
        nc.sync.dma_start(b1_sb[:], b1[:])
        gs_sb = cpool.tile([128, E], F32, name="gssb")
        nc.sync.dma_start(gs_sb[:], gsel[:])

        xf_pool = ctx.enter_context(tc.tile_pool(name="xf", bufs=2 * KC))
        xb_pool = ctx.enter_context(tc.tile_pool(name="xb", bufs=2 * KC))
        w1_pool = ctx.enter_context(tc.tile_pool(name="w1t", bufs=4))
        h_pool = ctx.enter_context(tc.tile_pool(name="ht", bufs=MC + 2))
        y_pool = ctx.enter_context(tc.tile_pool(name="yt", bufs=4))
        g_pool = ctx.enter_context(tc.tile_pool(name="gate8", bufs=3 * NSUB))
        s_pool = ctx.enter_context(tc.tile_pool(name="gate1", bufs=4 * NSUB))
        ph_pool = ctx.enter_context(tc.tile_pool(name="ph", bufs=3, space="PSUM"))
        py_pool = ctx.enter_context(tc.tile_pool(name="py", bufs=3, space="PSUM"))
        pr_pool = ctx.enter_context(tc.tile_pool(name="pr", bufs=2, space="PSUM"))

        # Dummy first PE instruction that depends only on wr_sb's DMA queue.
        # The first real matmul would otherwise need waits on two DMA-HW
        # queue semaphores (wr_sb + xft[0]) and the self-loading Matmult's
        # LDWEIGHTS slot supports only one sync wait (walrus setupSyncWait).
        prw = pr_pool.tile([128, E], F32, name="prr")
        nc.tensor.matmul(prw[0:E, :], wr_sb[:, 0:E], wr_sb[:, 0:E], start=True, stop=True)

        for g in range(NG):
            t0 = g * GT
            xft, xbt = [], []
            for k in range(KC):
                xt = xf_pool.tile([128, GT], F32, name="xft")
                nc.sync.dma_start(xt[:], xf[k, :, t0 : t0 + GT])
                xb = xb_pool.tile([128, GT], BF, name="xbt")
                nc.vector.tensor_copy(xb[:], xt[:])
                xft.append(xt)
                xbt.append(xb)

            # --- router: fp32 logits + top-2 renormalized gate -------------
            gcols = []
            for t in range(NSUB):
                ts128 = slice(t * 128, (t + 1) * 128)
                pr = pr_pool.tile([128, E], F32, name="prr")
                for k in range(KC):
                    nc.tensor.matmul(
                        pr[:],
                        xft[k][:, ts128],
                        wr_sb[:, k * E : (k + 1) * E],
                        start=(k == 0),
                        stop=(k == KC - 1),
                    )
                logits = g_pool.tile([128, E], F32, name="logits")
                nc.vector.tensor_add(logits[:], pr[:], br_sb[:])
                m1n = s_pool.tile([128, 1], F32, name="m1n")
                nc.vector.tensor_reduce(
                    m1n[:], logits[:], axis=AX.X, op=ALU.max, negate=True
                )
                # 1.0 where l == max (top-1), else 0.0
                mask1 = g_pool.tile([128, E], F32, name="mask1")
                nc.vector.tensor_scalar(
                    mask1[:], logits[:], m1n[:, 0:1], 0.0, ALU.add, ALU.is_ge
                )
                # logits with top-1 pushed to -inf, then (negated) max = -m2
                l2 = g_pool.tile([128, E], F32, name="l2")
                nc.vector.tensor_scalar(
                    l2[:], mask1[:], -1.0e30, None, ALU.mult
                )
                nc.vector.tensor_add(l2[:], l2[:], logits[:])
                m2n = s_pool.tile([128, 1], F32, name="m2n")
                nc.vector.tensor_reduce(
                    m2n[:], l2[:], axis=AX.X, op=ALU.max, negate=True
                )
                # z = exp(l - m1); keep only top-2 entries; renormalize
                z = g_pool.tile([128, E], F32, name="z")
                nc.scalar.activation(z[:], logits[:], AF.Exp, bias=m1n[:, 0:1])
                mask2 = g_pool.tile([128, E], F32, name="mask2")
                nc.vector.tensor_scalar(
                    mask2[:], logits[:], m2n[:, 0:1], 0.0, ALU.add, ALU.is_ge
                )
                zs = g_pool.tile([128, E], F32, name="zs")
                nc.vector.tensor_mul(zs[:], z[:], mask2[:])
                den = s_pool.tile([128, 1], F32, name="den")
                nc.vector.tensor_reduce(den[:], zs[:], axis=AX.X, op=ALU.add)
                rden = s_pool.tile([128, 1], F32, name="rden")
                nc.vector.reciprocal(rden[:], den[:])
                gate = g_pool.tile([128, E], F32, name="gatet")
                nc.vector.tensor_scalar(
                    gate[:], zs[:], rden[:, 0:1], None, ALU.mult
                )
                nc.sync.dma_start(gateo[t0 + t * 128 : t0 + (t + 1) * 128, :], gate[:])
                # this core's gate column via one-hot dot
                gsm = g_pool.tile([128, E], F32, name="gsm")
                nc.vector.tensor_mul(gsm[:], gate[:], gs_sb[:])
                gc = s_pool.tile([128, 1], F32, name="gc")
                nc.vector.tensor_reduce(gc[:], gsm[:], axis=AX.X, op=ALU.add)
                gcols.append(gc)

            # --- MM1: hT[m] = relu(sum_k W1[k,m].T @ xT[k] + b1[m]) --------
            hts = []
            for m in range(MC):
                w1t = w1_pool.tile([128, KC * 128], BF, name="w1t")
                nc.sync.dma_start(w1t[:], w1[:, m, :])
                ph = ph_pool.tile([128, GT], F32, name="ph")
                for k in range(KC):
                    nc.tensor.matmul(
                        ph[:],
                        w1t[:, k * 128 : (k + 1) * 128],
                        xbt[k][:],
                        start=(k == 0),
                        stop=(k == KC - 1),
                    )
                ht = h_pool.tile([128, GT], BF, name="ht")
                nc.scalar.activation(ht[:], ph[:], AF.Relu, bias=b1_sb[:, m : m + 1])
                hts.append(ht)

            # --- MM2 + gate scale + store ----------------------------------
            for t in range(NSUB):
                ts128 = slice(t * 128, (t + 1) * 128)
                for dh in range(DHALF):
                    py = py_pool.tile([128, 512], F32, name="py")
                    for m in range(MC):
                        nc.tensor.matmul(
                            py[:],
                            hts[m][:, ts128],
                            w2_sb[:, m * D + dh * 512 : m * D + (dh + 1) * 512],
                            start=(m == 0),
                            stop=(m == MC - 1),
                        )
                    yt = y_pool.tile([128, 512], F32, name="yt")
                    nc.scalar.mul(yt[:], py[:], gcols[t][:, 0:1])
                    nc.sync.dma_start(
                        outy[
                            t0 + t * 128 : t0 + (t + 1) * 128,
                            dh * 512 : (dh + 1) * 512,
                        ],
                        yt[:],
                    )

    _split_multi_waits(nc)
    return nc


# Walrus's setupSyncWait rejects engine instructions carrying more than one
# sync-wait command (observed for Matmult/Activation). Tile's scheduler can
# legitimately attach several semaphore waits to one instruction; split the
# extras onto same-engine InstNoOps inserted just before it (the same
# mechanism Tile uses for its own drains), preserving semantics exactly.
_SPLIT_SKIP: set = set()


def _split_multi_waits(nc: bass.Bass) -> None:
    n_split = 0
    for blk in nc.m.functions[0].blocks:
        insts = blk.instructions
        idx = 0
        while idx < len(insts):
            i = insts[idx]
            si = i.sync_info
            if (
                si is not None
                and len(si.on_wait) >= 2
                and type(i).__name__ not in _SPLIT_SKIP
            ):
                waits = list(si.on_wait)
                for w in waits[:-1]:
                    nop = mybir.InstNoOp(
                        name=nc.get_next_instruction_name(),
                        sync_info=mybir.SyncInfo(on_wait=[w], on_update=[]),
                        bass_nofuse=True,
                        engine=i.engine,
                    )
                    insts.insert(idx, nop)
                    idx += 1
                si.on_wait = [waits[-1]]
                n_split += 1
            idx += 1


def _prep_inputs(x, Wr, br, W1, b1, W2, b2):
    """Host-side sharding: rearrange/cast the full inputs into per-core maps."""
    xr = np.ascontiguousarray(
        x.reshape(NTOK, KC, 128).transpose(1, 2, 0)
    )  # [KC,128,NTOK] f32
    wrc = np.ascontiguousarray(
        Wr.reshape(KC, 128, E).transpose(1, 0, 2).reshape(128, KC * E)
    ).astype(np.float32)
    brc = np.ascontiguousarray(np.broadcast_to(br, (128, E))).astype(np.float32)
    in_maps = []
    for e in range(E):
        w1c = np.ascontiguousarray(
            W1[e].reshape(KC, 128, MC, 128).transpose(1, 2, 0, 3).reshape(128, MC, KC * 128)
        ).astype(BF16)
        w2c = np.ascontiguousarray(
            W2[e].reshape(MC, 128, D).transpose(1, 0, 2).reshape(128, MC * D)
        ).astype(BF16)
        b1c = np.ascontiguousarray(b1[e].reshape(MC, 128).T).astype(np.float32)
        onehot = np.zeros((E,), np.float32)
        onehot[e] = 1.0
        gselc = np.ascontiguousarray(np.broadcast_to(onehot, (128, E)))
        in_maps.append(
            {
                "xf": xr,
                "w1": w1c,
                "w2": w2c,
                "wr": wrc,
                "brt": brc,
                "b1": b1c,
                "gsel": gselc,
            }
        )
    return in_maps


def run_on_device(x, Wr, br, W1, b1, W2, b2, trace=False):
    """Returns (out [B,S,D] f32, BassKernelResults)."""
    if "nc" not in _CACHE:
        _CACHE["nc"] = _build_nc()
    nc = _CACHE["nc"]
    in_maps = _prep_inputs(x, Wr, br, W1, b1, W2, b2)
    res = run_bass_kernel_spmd(nc, in_maps, core_ids=list(range(E)), trace=trace)
    out = np.zeros((NTOK, D), np.float32)
    for r in res.results:
        out += r["outy"]
    out += res.results[0]["gateo"].astype(np.float32) @ b2.astype(np.float32)
    return out.reshape(B, S, D), res


def kernel(x, Wr, br, W1, b1, W2, b2):
    x = np.asarray(x, np.float32)
    Wr = np.asarray(Wr, np.float32)
    br = np.asarray(br, np.float32)
    W1 = np.asarray(W1, np.float32)
    b1 = np.asarray(b1, np.float32)
    W2 = np.asarray(W2, np.float32)
    b2 = np.asarray(b2, np.float32)
    out, _ = run_on_device(x, Wr, br, W1, b1, W2, b2, trace=False)
    return out
